# revision 26
# baseline (speedup 1.0000x reference)
"""CARAFE-Downsample Trainium2 kernel v4 (8 NeuronCores, batch-parallel).

v3 -> v4:
 - mask conv col-tiled 2-way: chunk j2's [25,512] psum slice sits at
   partition base 32*(j2%4); chunk pairs interleave round-robin so two
   accumulation chains run in different PE column groups concurrently.
 - input DMAs split (xcp in 8 row slices, xall in 5 slot slices) and
   ordered so chunk 0 / group 0 compute starts within a few us.
 - products via per-block tensor_scalar (DVE 4x mode) / activation-scale
   (ACT) / gpsimd tensor_scalar, reading per-partition weights straight
   from w3f; the pair-replicated w3g tile is gone.
 - m2 is one [128,1026] tile (group g at rows 32g); exp bias/scale and
   the transpose identity are host-replicated at partition offsets
   0/32/64/96.
 - per-group output DMAs (4 x 0.5 MB) instead of 2 x 1 MB.
"""

import numpy as np
import ml_dtypes

import concourse.bass as bass
import concourse.bacc as bacc
import concourse.tile as tile
from concourse import mybir
from concourse.bass_utils import run_bass_kernel_spmd

B, C, H, W = 8, 256, 128, 128
CC, KK, HP, WP, NB = 64, 5, 64, 64, 32
NCORES = 8
NG = 4          # groups
GB = 8          # blocks per group
XCPL = 16900    # fp8 plane stride (130*130)

BF16 = mybir.dt.bfloat16
F32 = mybir.dt.float32
FP8 = mybir.dt.float8e4
NPBF = ml_dtypes.bfloat16
NPF8 = ml_dtypes.float8_e4m3

_CPDJ = [(0, -1), (1, -1), (0, 0), (1, 0), (0, 1)]
_OHDH = [(0, -1), (1, -1), (0, 0), (1, 0), (0, 1)]
_VAR = {-1: 1, 0: 0, 1: 2}

# ---- tap assignment: v 18, a 4, g 3 (v3.2-best) ----
TAP_MODE = {}
_dj0 = [(i, j) for j in (2, 3) for i in range(5)]    # 10 taps, dj=0
_djm = [(i, j) for j in (0, 1) for i in range(5)]    # 10 taps, dj=-1
_djp = [(i, 4) for i in range(5)]                    # 5 taps,  dj=+1
for t in _dj0:
    TAP_MODE[t] = "v"
for t in _djm[:5]:
    TAP_MODE[t] = "v"
for t in _djm[5:9]:
    TAP_MODE[t] = "a"
TAP_MODE[_djm[9]] = "g"
for t in _djp[:2]:
    TAP_MODE[t] = "v"
for t in _djp[2:]:
    TAP_MODE[t] = "g"


def _slot(kk, oh, cp):
    return ((kk + 1) * 2 + oh) * 2 + cp


def _build_nc():
    nc = bacc.Bacc(None, target_bir_lowering=False, debug=False)

    xall_d = nc.declare_dram_parameter("xall", [128, 136, C], BF16, isOutput=False)
    xcp_d = nc.declare_dram_parameter("xcp", [128, 130, 2, 130], FP8,
                                      isOutput=False)
    w2_d = nc.declare_dram_parameter("w2", [128, 18, 25], FP8, isOutput=False)
    b2_d = nc.declare_dram_parameter("b2", [128, 1], F32, isOutput=False)
    sc_d = nc.declare_dram_parameter("sc", [128, 1], F32, isOutput=False)
    id_d = nc.declare_dram_parameter("idn", [128, 25], BF16, isOutput=False)
    shm_d = nc.declare_dram_parameter("shm", [128, 3, 128], BF16, isOutput=False)
    out_d = nc.declare_dram_parameter("out", [128, NB * C], BF16, isOutput=True)

    taps = [(i, j) for i in range(5) for j in range(5)]
    mm_taps = ([t for t in taps if _CPDJ[t[1]][1] == -1]
               + [t for t in taps if _CPDJ[t[1]][1] == 1]
               + [t for t in taps if _CPDJ[t[1]][1] == 0])

    with tile.TileContext(nc) as tc:
        with (
            tc.tile_pool(name="consts", bufs=1) as consts,
            tc.tile_pool(name="xbig", bufs=1) as xbig,
            tc.tile_pool(name="psM", bufs=2, space="PSUM") as psM,
            tc.tile_pool(name="psW", bufs=2, space="PSUM") as psW,
            tc.tile_pool(name="psP", bufs=4, space="PSUM") as psP,
            tc.tile_pool(name="wrep", bufs=4) as wrep,
            tc.tile_pool(name="wsb", bufs=4) as wsb,
            tc.tile_pool(name="prodp", bufs=10) as prodp,
        ):
            # ---- const DMAs (small, first) ----
            w2_sb = consts.tile([128, 18, 25], FP8)
            nc.sync.dma_start(out=w2_sb, in_=w2_d[:, :, :])
            shm_sb = consts.tile([128, 3, 128], BF16)
            nc.sync.dma_start(out=shm_sb, in_=shm_d[:, :, :])
            b2_sb = consts.tile([128, 1], F32)
            nc.sync.dma_start(out=b2_sb, in_=b2_d[:, :])
            sc_sb = consts.tile([128, 1], F32)
            nc.sync.dma_start(out=sc_sb, in_=sc_d[:, :])
            id_sb = consts.tile([128, 25], BF16)
            nc.sync.dma_start(out=id_sb, in_=id_d[:, :])

            # ---- big inputs: one tile per slice (exact dependencies),
            # halo rows/slots duplicated across slices. `eng` picks the
            # issuing engine stream so later transfers are submitted
            # mid-kernel instead of competing with the critical head. ----
            xcp_t = [xbig.tile([128, (18 if j == 7 else 17), 2, 130], FP8,
                               name=f"xcp{j}") for j in range(8)]
            xall_t = [xbig.tile([128, 40, C], BF16, name=f"xallg{g}")
                      for g in range(NG)]

            def dma_xcp(j, eng=None):
                r0 = 16 * j
                nr = 18 if j == 7 else 17
                (eng or nc.sync).dma_start(
                    out=xcp_t[j][:, :, :, :],
                    in_=xcp_d[:, r0:r0 + nr, :, :])

            def dma_xall(g, eng=None):
                (eng or nc.sync).dma_start(out=xall_t[g][:, :, :],
                                           in_=xall_d[:, 32 * g:32 * g + 40, :])

            outst0 = xbig.tile([128, 16 * C], BF16)
            outst1 = xbig.tile([128, 16 * C], BF16)

            m2all = xbig.tile([128, 1026], BF16)
            nc.vector.memset(m2all[:, 0:1], 1.0)
            nc.vector.memset(m2all[:, 1025:1026], 1.0)

            # ---- mask conv chunk (col-tiled by j2%4) ----
            def emit_chunk_pair(ja, jb):
                pms = {}
                for j2 in (ja, jb):
                    pms[j2] = psM.tile([128, 512], F32, name=f"pm{j2}",
                                       tag="pm")
                for ti in range(18):
                    di, dj2 = divmod(ti // 2, 3)
                    e = ti % 2
                    for j2 in (ja, jb):
                        cg = 32 * (j2 % 4)
                        xs = xcp_t[j2]
                        off = di * 260 + e * 130 + dj2
                        rhs = bass.AP(
                            tensor=xs.tensor,
                            offset=xs.offset + off,
                            ap=[xs.ap[0], [520, 8], [2, 64]],
                        )
                        nc.tensor.matmul(pms[j2][cg:cg + 25, :],
                                         lhsT=w2_sb[:, (di * 3 + dj2) * 2 + e, :],
                                         rhs=rhs, start=(ti == 0),
                                         stop=(ti == 17),
                                         tile_position=(0, cg))
                for j2 in (ja, jb):
                    cg = 32 * (j2 % 4)
                    half = 0 if j2 < 4 else 1
                    base = 1 + half * 64
                    dstv = m2all[cg:cg + 25, :]
                    dst = bass.AP(tensor=dstv.tensor, offset=dstv.offset + base,
                                  ap=[dstv.ap[0], [128, 8], [1, 64]])
                    nc.scalar.activation(
                        out=dst,
                        in_=pms[j2][cg:cg + 25, :].rearrange(
                            "p (r n) -> p r n", n=64),
                        func=mybir.ActivationFunctionType.Exp,
                        bias=b2_sb[cg:cg + 25, :], scale=sc_sb[cg:cg + 25, :])

            # ---- weights for group g ----
            w3f_g, w3g_g = {}, {}

            def emit_weights(g):
                r0 = 32 * g
                m2v = m2all[r0:r0 + 25, :]
                idv = id_sb[r0:r0 + 25, :]
                w3f = wrep.tile([128, GB, 3, 25], F32, name=f"w3f{g}",
                                tag="w3f")
                w3g = wrep.tile([128, GB, 3, 25, 2], BF16, name=f"w3g{g}",
                                tag="w3g")
                for b2i in range(GB // 2):
                    kl = 2 * b2i
                    pw = psW.tile([128, 2, 128], BF16, name=f"pw{g}_{b2i}",
                                  tag="pw")
                    for bb in range(2):
                        nc.tensor.transpose(
                            pw[:, bb, 0:25],
                            m2v[:, 1 + (kl + bb) * 128:1 + (kl + bb + 1) * 128],
                            idv, tile_position=(r0, 0))
                        nc.tensor.transpose(
                            pw[:, bb, 32:57],
                            m2v[:, 2 + (kl + bb) * 128:2 + (kl + bb + 1) * 128],
                            idv, tile_position=(r0, 0))
                        nc.tensor.transpose(
                            pw[:, bb, 64:89],
                            m2v[:, (kl + bb) * 128:(kl + bb) * 128 + 128],
                            idv, tile_position=(r0, 0))
                    v6 = bass.AP(tensor=pw.tensor, offset=pw.offset,
                                 ap=[pw.ap[0], [128, 2], [32, 3], [1, 25]])
                    r6 = wsb.tile([128, 6], F32, name=f"r6{g}_{b2i}", tag="r6")
                    r6v = bass.AP(tensor=r6.tensor, offset=r6.offset,
                                  ap=[r6.ap[0], [3, 2], [1, 3]])
                    nc.vector.tensor_reduce(out=r6v, in_=v6,
                                            axis=mybir.AxisListType.X,
                                            op=mybir.AluOpType.add)
                    nc.vector.reciprocal(out=r6, in_=r6)
                    dstf = bass.AP(tensor=w3f.tensor,
                                   offset=w3f.offset + 2 * b2i * 75,
                                   ap=[w3f.ap[0], [75, 2], [25, 3], [1, 25]])
                    in1f = bass.AP(tensor=r6.tensor, offset=r6.offset,
                                   ap=[r6.ap[0], [3, 2], [1, 3], [0, 25]])
                    nc.vector.tensor_tensor(out=dstf, in0=v6, in1=in1f,
                                            op=mybir.AluOpType.mult)
                rep_dst = bass.AP(tensor=w3g.tensor, offset=w3g.offset,
                                  ap=[w3g.ap[0], [2, 600], [1, 2]])
                rep_src = bass.AP(tensor=w3f.tensor, offset=w3f.offset,
                                  ap=[w3f.ap[0], [1, 600], [0, 2]])
                nc.scalar.copy(out=rep_dst, in_=rep_src)
                w3f_g[g], w3g_g[g] = w3f, w3g

            # ---- reassembly group ----
            def emit_group(g):
                k0 = g * GB
                w3f, w3g = w3f_g[g], w3g_g[g]

                xa = xall_t[g]

                def emit_product(tap, mode):
                    i, j = tap
                    cp, dj = _CPDJ[j]
                    oh, dh = _OHDH[i]
                    t = i * 5 + j
                    var = _VAR[dj]
                    s0 = _slot(dh, oh, cp)   # slot local to this group tile
                    pt = prodp.tile([128, GB, C], BF16, name=f"p{g}_{t}",
                                    tag="prod")
                    if mode == "a":
                        for b in range(GB):
                            scb = bass.AP(
                                tensor=w3f.tensor,
                                offset=w3f.offset + b * 75 + var * 25 + t,
                                ap=[w3f.ap[0], [1, 1]])
                            nc.scalar.activation(
                                out=pt[:, b, :],
                                in_=xa[:, s0 + 4 * b, :],
                                func=mybir.ActivationFunctionType.Copy,
                                scale=scb)
                        return pt
                    in0 = bass.AP(tensor=xa.tensor,
                                  offset=xa.offset + s0 * C,
                                  ap=[xa.ap[0], [4 * C, GB],
                                      [2, C // 2], [1, 2]])
                    in1 = bass.AP(tensor=w3g.tensor,
                                  offset=w3g.offset + var * 50 + t * 2,
                                  ap=[w3g.ap[0], [150, GB], [0, C // 2],
                                      [1, 2]])
                    outp = bass.AP(tensor=pt.tensor, offset=pt.offset,
                                   ap=[pt.ap[0], [C, GB], [2, C // 2], [1, 2]])
                    if mode == "g":
                        nc.gpsimd.tensor_tensor(out=outp, in0=in0, in1=in1,
                                                op=mybir.AluOpType.mult)
                    else:
                        nc.vector.tensor_tensor(out=outp, in0=in0, in1=in1,
                                                op=mybir.AluOpType.mult)
                    return pt

                po_list = [psP.tile([128, 512], F32, name=f"po{g}_{pr}",
                                    tag="po") for pr in range(GB // 2)]
                started = [False] * (GB // 2)
                # slow producers (GPSIMD) build their products up front
                early = {}
                for tap in mm_taps:
                    if TAP_MODE[tap] == "g":
                        early[tap] = emit_product(tap, "g")
                units = [(_VAR[_CPDJ[t[1]][1]], t) for t in mm_taps]
                units.sort(key=lambda u: (u[1] in early,
                                          TAP_MODE[u[1]] == "a",
                                          u[0] != 0, u[0]))
                # spread ACT products: one after every 3rd other unit
                a_units = [u for u in units if TAP_MODE[u[1]] == "a"]
                o_units = [u for u in units if TAP_MODE[u[1]] != "a"]
                units = []
                ai = 0
                for n, u in enumerate(o_units):
                    units.append(u)
                    if n % 3 == 2 and ai < len(a_units):
                        units.append(a_units[ai])
                        ai += 1
                units.extend(a_units[ai:])
                for nu, (var, tap) in enumerate(units):
                    pt = (early[tap] if tap in early
                          else emit_product(tap, TAP_MODE[tap]))
                    last = nu == len(units) - 1
                    for pr in range(GB // 2):
                        nc.tensor.matmul(po_list[pr], lhsT=shm_sb[:, var, :],
                                         rhs=pt[:, 2 * pr:2 * pr + 2, :],
                                         start=not started[pr], stop=last)
                        started[pr] = True

                st = outst0 if k0 < 16 else outst1
                for pr in range(GB // 2):
                    po = po_list[pr]
                    k = k0 + 2 * pr
                    dst = st[:, (k % 16) * C:(k % 16 + 2) * C]
                    nc.scalar.copy(out=dst, in_=po)
                nc.sync.dma_start(
                    out=out_d[:, k0 * C:(k0 + GB) * C],
                    in_=st[:, (k0 % 16) * C:((k0 % 16) + GB) * C])

            # ---- schedule ----
            # immediate (sync queue): the critical head — chunks 0,1,4,5
            # and group 0's x data (~4.9 MB).
            dma_xcp(0)
            dma_xcp(1)
            dma_xcp(4)
            dma_xcp(5)
            dma_xall(0)

            emit_chunk_pair(0, 1)
            # deferred transfers: submitted from the ACT stream after the
            # exps above, keeping HBM clear for the critical head.
            dma_xcp(2, eng=nc.scalar)
            dma_xcp(3, eng=nc.scalar)
            emit_chunk_pair(4, 5)
            dma_xall(1, eng=nc.scalar)
            emit_weights(0)
            emit_weights(1)
            dma_xall(2, eng=nc.scalar)
            emit_group(0)
            emit_chunk_pair(2, 3)
            dma_xcp(6, eng=nc.scalar)
            dma_xcp(7, eng=nc.scalar)
            dma_xall(3, eng=nc.scalar)
            emit_chunk_pair(6, 7)
            emit_weights(2)
            emit_weights(3)
            emit_group(1)
            emit_group(2)
            emit_group(3)

    nc.compile()
    return nc


_NC_CACHE = None
LAST_RESULTS = None


def _get_nc():
    global _NC_CACHE
    if _NC_CACHE is None:
        _NC_CACHE = _build_nc()
    return _NC_CACHE


def _host_prep(x, w_comp, b_comp, w_enc, b_enc, power_p):
    pe = float(np.exp(np.float64(power_p)))

    xb = x.astype(NPBF)
    X_all = np.zeros((B, 128, 136, C), dtype=NPBF)
    for oh in range(2):
        for cp in range(2):
            g = xb[:, :, :, cp::2]
            for h in range(2):
                kks = [kk for kk in range(-1, 33)
                       if 0 <= 2 * (kk + 32 * h) + oh < H]
                rows = [2 * (kk + 32 * h) + oh for kk in kks]
                slots = [_slot(kk, oh, cp) for kk in kks]
                sub = g[:, :, rows, :].transpose(0, 3, 2, 1)
                X_all[:, 64 * h:64 * h + 64, slots, :] = sub

    xpad = np.zeros((B, 128, 130, 2, 130), dtype=NPF8)
    xp = np.pad(x, ((0, 0), (0, 0), (1, 1), (1, 1))).astype(NPF8)
    for e in range(2):
        xpad[:, :, :, e, :] = xp[:, e * 128:(e + 1) * 128]

    wc = w_comp[:, :, 0, 0].astype(np.float64)
    W2 = np.einsum('tkij,kc->tijc', w_enc.astype(np.float64), wc)
    bias2 = b_enc.astype(np.float64) + \
        w_enc.astype(np.float64).sum(axis=(2, 3)) @ b_comp.astype(np.float64)
    amax = max(np.abs(W2).max(), 1e-30)
    SCALE = 2.0 ** np.floor(np.log2(192.0 / amax))
    w2s = np.zeros((128, 18, 25), dtype=NPF8)
    for di in range(3):
        for dj in range(3):
            for e in range(2):
                w2s[:, (di * 3 + dj) * 2 + e, :] = \
                    (W2[:, di, dj, e * 128:(e + 1) * 128].T * SCALE).astype(NPF8)
    b2 = np.zeros((128, 1), dtype=np.float32)
    sc = np.zeros((128, 1), dtype=np.float32)
    idn = np.zeros((128, 25), dtype=NPBF)
    for jo in range(4):
        b2[32 * jo:32 * jo + 25, 0] = (pe * bias2).astype(np.float32)
        sc[32 * jo:32 * jo + 25, 0] = pe / SCALE
        idn[32 * jo + np.arange(25), np.arange(25)] = 1

    shm = np.zeros((128, 3, 128), dtype=NPBF)
    shm[np.arange(128), 0, np.arange(128)] = 1
    shm[np.arange(127), 1, np.arange(127) + 1] = 1
    shm[63, 1, 64] = 0
    shm[np.arange(1, 128), 2, np.arange(1, 128) - 1] = 1
    shm[64, 2, 63] = 0

    in_maps = []
    for b in range(B):
        in_maps.append({
            "xall": np.ascontiguousarray(X_all[b]),
            "xcp": np.ascontiguousarray(xpad[b]),
            "w2": w2s, "b2": b2, "sc": sc, "idn": idn, "shm": shm,
        })
    return in_maps


def kernel(x, w_comp, b_comp, w_enc, b_enc, power_p):
    x = np.asarray(x, dtype=np.float32)
    in_maps = _host_prep(x, np.asarray(w_comp), np.asarray(b_comp),
                         np.asarray(w_enc), np.asarray(b_enc),
                         np.asarray(power_p))
    nc = _get_nc()
    res = run_bass_kernel_spmd(nc, in_maps, list(range(NCORES)))
    global LAST_RESULTS
    LAST_RESULTS = res
    outs = np.stack([np.asarray(res.results[i]["out"]) for i in range(NCORES)])
    o = outs.reshape(B, 2, 64, NB, C).astype(np.float32)
    out = np.zeros((B, C, HP, WP), dtype=np.float32)
    for h in range(2):
        out[:, :, 32 * h:32 * h + 32, :] = o[:, h].transpose(0, 3, 2, 1)
    return np.ascontiguousarray(out)


# revision 27
# speedup vs baseline: 1.0422x; 1.0422x over previous
"""CARAFE-Downsample Trainium2 kernel v4 (8 NeuronCores, batch-parallel).

v3 -> v4:
 - mask conv col-tiled 2-way: chunk j2's [25,512] psum slice sits at
   partition base 32*(j2%4); chunk pairs interleave round-robin so two
   accumulation chains run in different PE column groups concurrently.
 - input DMAs split (xcp in 8 row slices, xall in 5 slot slices) and
   ordered so chunk 0 / group 0 compute starts within a few us.
 - products via per-block tensor_scalar (DVE 4x mode) / activation-scale
   (ACT) / gpsimd tensor_scalar, reading per-partition weights straight
   from w3f; the pair-replicated w3g tile is gone.
 - m2 is one [128,1026] tile (group g at rows 32g); exp bias/scale and
   the transpose identity are host-replicated at partition offsets
   0/32/64/96.
 - per-group output DMAs (4 x 0.5 MB) instead of 2 x 1 MB.
"""

import numpy as np
import ml_dtypes

import concourse.bass as bass
import concourse.bacc as bacc
import concourse.tile as tile
from concourse import mybir
from concourse.bass_utils import run_bass_kernel_spmd

B, C, H, W = 8, 256, 128, 128
CC, KK, HP, WP, NB = 64, 5, 64, 64, 32
NCORES = 8
NG = 4          # groups
GB = 8          # blocks per group
XCPL = 16900    # fp8 plane stride (130*130)

BF16 = mybir.dt.bfloat16
F32 = mybir.dt.float32
FP8 = mybir.dt.float8e4
NPBF = ml_dtypes.bfloat16
NPF8 = ml_dtypes.float8_e4m3

_CPDJ = [(0, -1), (1, -1), (0, 0), (1, 0), (0, 1)]
_OHDH = [(0, -1), (1, -1), (0, 0), (1, 0), (0, 1)]
_VAR = {-1: 1, 0: 0, 1: 2}

# ---- tap assignment: v 18, a 4, g 3 (v3.2-best) ----
TAP_MODE = {}
_dj0 = [(i, j) for j in (2, 3) for i in range(5)]    # 10 taps, dj=0
_djm = [(i, j) for j in (0, 1) for i in range(5)]    # 10 taps, dj=-1
_djp = [(i, 4) for i in range(5)]                    # 5 taps,  dj=+1
for t in _dj0:
    TAP_MODE[t] = "v"
for t in _djm[:5]:
    TAP_MODE[t] = "v"
for t in _djm[5:9]:
    TAP_MODE[t] = "a"
TAP_MODE[_djm[9]] = "g"
for t in _djp[:3]:
    TAP_MODE[t] = "v"
for t in _djp[3:]:
    TAP_MODE[t] = "g"


def _slot(kk, oh, cp):
    return ((kk + 1) * 2 + oh) * 2 + cp


def _build_nc():
    nc = bacc.Bacc(None, target_bir_lowering=False, debug=False)

    xall_d = nc.declare_dram_parameter("xall", [128, 136, C], BF16, isOutput=False)
    xcp_d = nc.declare_dram_parameter("xcp", [128, 130, 2, 130], FP8,
                                      isOutput=False)
    w2_d = nc.declare_dram_parameter("w2", [128, 18, 25], FP8, isOutput=False)
    b2_d = nc.declare_dram_parameter("b2", [128, 1], F32, isOutput=False)
    sc_d = nc.declare_dram_parameter("sc", [128, 1], F32, isOutput=False)
    id_d = nc.declare_dram_parameter("idn", [128, 25], BF16, isOutput=False)
    shm_d = nc.declare_dram_parameter("shm", [128, 3, 128], BF16, isOutput=False)
    out_d = nc.declare_dram_parameter("out", [128, NB * C], BF16, isOutput=True)

    taps = [(i, j) for i in range(5) for j in range(5)]
    mm_taps = ([t for t in taps if _CPDJ[t[1]][1] == -1]
               + [t for t in taps if _CPDJ[t[1]][1] == 1]
               + [t for t in taps if _CPDJ[t[1]][1] == 0])

    with tile.TileContext(nc) as tc:
        with (
            tc.tile_pool(name="consts", bufs=1) as consts,
            tc.tile_pool(name="xbig", bufs=1) as xbig,
            tc.tile_pool(name="psM", bufs=2, space="PSUM") as psM,
            tc.tile_pool(name="psW", bufs=2, space="PSUM") as psW,
            tc.tile_pool(name="psP", bufs=4, space="PSUM") as psP,
            tc.tile_pool(name="wrep", bufs=4) as wrep,
            tc.tile_pool(name="wsb", bufs=4) as wsb,
            tc.tile_pool(name="prodp", bufs=10) as prodp,
        ):
            # ---- const DMAs (small, first) ----
            w2_sb = consts.tile([128, 18, 25], FP8)
            nc.sync.dma_start(out=w2_sb, in_=w2_d[:, :, :])
            shm_sb = consts.tile([128, 3, 128], BF16)
            nc.sync.dma_start(out=shm_sb, in_=shm_d[:, :, :])
            b2_sb = consts.tile([128, 1], F32)
            nc.sync.dma_start(out=b2_sb, in_=b2_d[:, :])
            sc_sb = consts.tile([128, 1], F32)
            nc.sync.dma_start(out=sc_sb, in_=sc_d[:, :])
            id_sb = consts.tile([128, 25], BF16)
            nc.sync.dma_start(out=id_sb, in_=id_d[:, :])

            # ---- big inputs: one tile per slice (exact dependencies),
            # halo rows/slots duplicated across slices. `eng` picks the
            # issuing engine stream so later transfers are submitted
            # mid-kernel instead of competing with the critical head. ----
            xcp_t = [xbig.tile([128, (34 if p == 3 else 33), 2, 130], FP8,
                               name=f"xcpp{p}") for p in range(4)]
            xall_t = [xbig.tile([128, 40, C], BF16, name=f"xallg{g}")
                      for g in range(NG)]

            def dma_xcp(p, eng=None):
                r0 = 32 * p
                nr = 34 if p == 3 else 33
                (eng or nc.sync).dma_start(
                    out=xcp_t[p][:, :, :, :],
                    in_=xcp_d[:, r0:r0 + nr, :, :])

            def dma_xall(g, eng=None):
                (eng or nc.sync).dma_start(out=xall_t[g][:, :, :],
                                           in_=xall_d[:, 32 * g:32 * g + 40, :])

            outst0 = xbig.tile([128, 16 * C], BF16)
            outst1 = xbig.tile([128, 16 * C], BF16)

            m2all = xbig.tile([128, 1026], BF16)
            nc.vector.memset(m2all[:, 0:1], 1.0)
            nc.vector.memset(m2all[:, 1025:1026], 1.0)

            # ---- mask conv chunk (col-tiled by j2%4) ----
            def emit_chunk_pair(ja, jb):
                pm = psM.tile([128, 512], F32, name=f"pm{ja}_{jb}", tag="pm")
                for ti in range(18):
                    di, dj2 = divmod(ti // 2, 3)
                    e = ti % 2
                    for j2 in (ja, jb):
                        cg = 32 * (j2 % 4)
                        xs = xcp_t[j2 // 2]
                        rl = 16 * j2 - 32 * (j2 // 2)
                        off = (rl + di) * 260 + e * 130 + dj2
                        rhs = bass.AP(
                            tensor=xs.tensor,
                            offset=xs.offset + off,
                            ap=[xs.ap[0], [520, 8], [2, 64]],
                        )
                        nc.tensor.matmul(pm[cg:cg + 25, :],
                                         lhsT=w2_sb[:, (di * 3 + dj2) * 2 + e, :],
                                         rhs=rhs, start=(ti == 0),
                                         stop=(ti == 17),
                                         tile_position=(0, cg),
                                         skip_group_check=True)
                for j2 in (ja, jb):
                    cg = 32 * (j2 % 4)
                    half = 0 if j2 < 4 else 1
                    base = 1 + half * 64
                    dstv = m2all[cg:cg + 25, :]
                    dst = bass.AP(tensor=dstv.tensor, offset=dstv.offset + base,
                                  ap=[dstv.ap[0], [128, 8], [1, 64]])
                    nc.scalar.activation(
                        out=dst,
                        in_=pm[cg:cg + 25, :].rearrange(
                            "p (r n) -> p r n", n=64),
                        func=mybir.ActivationFunctionType.Exp,
                        bias=b2_sb[cg:cg + 25, :], scale=sc_sb[cg:cg + 25, :])

            # ---- weights for group g ----
            w3f_g, w3g_g = {}, {}

            def emit_weights(g):
                r0 = 32 * g
                m2v = m2all[r0:r0 + 25, :]
                idv = id_sb[r0:r0 + 25, :]
                w3f = wrep.tile([128, GB, 3, 25], F32, name=f"w3f{g}",
                                tag="w3f")
                w3g = wrep.tile([128, GB, 3, 25, 2], BF16, name=f"w3g{g}",
                                tag="w3g")
                for b2i in range(GB // 2):
                    kl = 2 * b2i
                    pw = psW.tile([128, 2, 128], BF16, name=f"pw{g}_{b2i}",
                                  tag="pw")
                    for bb in range(2):
                        nc.tensor.transpose(
                            pw[:, bb, 0:25],
                            m2v[:, 1 + (kl + bb) * 128:1 + (kl + bb + 1) * 128],
                            idv, tile_position=(r0, 0))
                        nc.tensor.transpose(
                            pw[:, bb, 32:57],
                            m2v[:, 2 + (kl + bb) * 128:2 + (kl + bb + 1) * 128],
                            idv, tile_position=(r0, 0))
                        nc.tensor.transpose(
                            pw[:, bb, 64:89],
                            m2v[:, (kl + bb) * 128:(kl + bb) * 128 + 128],
                            idv, tile_position=(r0, 0))
                    v6 = bass.AP(tensor=pw.tensor, offset=pw.offset,
                                 ap=[pw.ap[0], [128, 2], [32, 3], [1, 25]])
                    r6 = wsb.tile([128, 6], F32, name=f"r6{g}_{b2i}", tag="r6")
                    r6v = bass.AP(tensor=r6.tensor, offset=r6.offset,
                                  ap=[r6.ap[0], [3, 2], [1, 3]])
                    nc.vector.tensor_reduce(out=r6v, in_=v6,
                                            axis=mybir.AxisListType.X,
                                            op=mybir.AluOpType.add)
                    nc.vector.reciprocal(out=r6, in_=r6)
                    dstf = bass.AP(tensor=w3f.tensor,
                                   offset=w3f.offset + 2 * b2i * 75,
                                   ap=[w3f.ap[0], [75, 2], [25, 3], [1, 25]])
                    in1f = bass.AP(tensor=r6.tensor, offset=r6.offset,
                                   ap=[r6.ap[0], [3, 2], [1, 3], [0, 25]])
                    nc.vector.tensor_tensor(out=dstf, in0=v6, in1=in1f,
                                            op=mybir.AluOpType.mult)
                rep_dst = bass.AP(tensor=w3g.tensor, offset=w3g.offset,
                                  ap=[w3g.ap[0], [2, 600], [1, 2]])
                rep_src = bass.AP(tensor=w3f.tensor, offset=w3f.offset,
                                  ap=[w3f.ap[0], [1, 600], [0, 2]])
                nc.vector.tensor_copy(out=rep_dst, in_=rep_src)
                w3f_g[g], w3g_g[g] = w3f, w3g

            # ---- reassembly group ----
            def emit_group(g):
                k0 = g * GB
                w3f, w3g = w3f_g[g], w3g_g[g]

                xa = xall_t[g]

                def emit_product(tap, mode):
                    i, j = tap
                    cp, dj = _CPDJ[j]
                    oh, dh = _OHDH[i]
                    t = i * 5 + j
                    var = _VAR[dj]
                    s0 = _slot(dh, oh, cp)   # slot local to this group tile
                    pt = prodp.tile([128, GB, C], BF16, name=f"p{g}_{t}",
                                    tag="prod")
                    if mode == "a":
                        for b in range(GB):
                            scb = bass.AP(
                                tensor=w3f.tensor,
                                offset=w3f.offset + b * 75 + var * 25 + t,
                                ap=[w3f.ap[0], [1, 1]])
                            nc.scalar.activation(
                                out=pt[:, b, :],
                                in_=xa[:, s0 + 4 * b, :],
                                func=mybir.ActivationFunctionType.Copy,
                                scale=scb)
                        return pt
                    in0 = bass.AP(tensor=xa.tensor,
                                  offset=xa.offset + s0 * C,
                                  ap=[xa.ap[0], [4 * C, GB],
                                      [2, C // 2], [1, 2]])
                    in1 = bass.AP(tensor=w3g.tensor,
                                  offset=w3g.offset + var * 50 + t * 2,
                                  ap=[w3g.ap[0], [150, GB], [0, C // 2],
                                      [1, 2]])
                    outp = bass.AP(tensor=pt.tensor, offset=pt.offset,
                                   ap=[pt.ap[0], [C, GB], [2, C // 2], [1, 2]])
                    if mode == "g":
                        nc.gpsimd.tensor_tensor(out=outp, in0=in0, in1=in1,
                                                op=mybir.AluOpType.mult)
                    else:
                        nc.vector.tensor_tensor(out=outp, in0=in0, in1=in1,
                                                op=mybir.AluOpType.mult)
                    return pt

                po_list = [psP.tile([128, 512], F32, name=f"po{g}_{pr}",
                                    tag="po") for pr in range(GB // 2)]
                started = [False] * (GB // 2)
                # slow producers (GPSIMD) build their products up front
                early = {}
                for tap in mm_taps:
                    if TAP_MODE[tap] == "g":
                        early[tap] = emit_product(tap, "g")
                units = [(_VAR[_CPDJ[t[1]][1]], t) for t in mm_taps]
                units.sort(key=lambda u: (u[1] in early,
                                          TAP_MODE[u[1]] == "a",
                                          u[0] != 0, u[0]))
                # spread ACT products: one after every 3rd other unit
                a_units = [u for u in units if TAP_MODE[u[1]] == "a"]
                o_units = [u for u in units if TAP_MODE[u[1]] != "a"]
                units = []
                ai = 0
                for n, u in enumerate(o_units):
                    units.append(u)
                    if n % 3 == 2 and ai < len(a_units):
                        units.append(a_units[ai])
                        ai += 1
                units.extend(a_units[ai:])
                for nu, (var, tap) in enumerate(units):
                    pt = (early[tap] if tap in early
                          else emit_product(tap, TAP_MODE[tap]))
                    last = nu == len(units) - 1
                    for pr in range(GB // 2):
                        nc.tensor.matmul(po_list[pr], lhsT=shm_sb[:, var, :],
                                         rhs=pt[:, 2 * pr:2 * pr + 2, :],
                                         start=not started[pr], stop=last)
                        started[pr] = True

                st = outst0 if k0 < 16 else outst1
                for pr in range(GB // 2):
                    po = po_list[pr]
                    k = k0 + 2 * pr
                    dst = st[:, (k % 16) * C:(k % 16 + 2) * C]
                    nc.scalar.copy(out=dst, in_=po)
                nc.sync.dma_start(
                    out=out_d[:, k0 * C:(k0 + GB) * C],
                    in_=st[:, (k0 % 16) * C:((k0 % 16) + GB) * C])

            # ---- schedule ----
            # immediate (sync queue): exactly 8 transfers incl. consts --
            # one per DMA completion lane, so no head transfer queues
            # behind another.
            dma_xcp(0)   # chunks 0,1
            dma_xcp(2)   # chunks 4,5
            dma_xall(0)

            emit_chunk_pair(0, 1)
            # deferred transfers: submitted from the ACT stream after the
            # exps above, keeping HBM clear for the critical head.
            dma_xcp(1, eng=nc.scalar)   # chunks 2,3
            dma_xcp(3, eng=nc.scalar)   # chunks 6,7
            emit_chunk_pair(4, 5)
            dma_xall(1, eng=nc.scalar)
            emit_weights(0)
            emit_weights(1)
            emit_group(0)
            emit_chunk_pair(2, 3)
            dma_xall(2, eng=nc.scalar)
            emit_chunk_pair(6, 7)
            dma_xall(3, eng=nc.scalar)
            emit_weights(2)
            emit_weights(3)
            emit_group(1)
            emit_group(2)
            emit_group(3)

    nc.compile()
    return nc


_NC_CACHE = None
LAST_RESULTS = None


def _get_nc():
    global _NC_CACHE
    if _NC_CACHE is None:
        _NC_CACHE = _build_nc()
    return _NC_CACHE


def _host_prep(x, w_comp, b_comp, w_enc, b_enc, power_p):
    pe = float(np.exp(np.float64(power_p)))

    xb = x.astype(NPBF)
    X_all = np.zeros((B, 128, 136, C), dtype=NPBF)
    for oh in range(2):
        for cp in range(2):
            g = xb[:, :, :, cp::2]
            for h in range(2):
                kks = [kk for kk in range(-1, 33)
                       if 0 <= 2 * (kk + 32 * h) + oh < H]
                rows = [2 * (kk + 32 * h) + oh for kk in kks]
                slots = [_slot(kk, oh, cp) for kk in kks]
                sub = g[:, :, rows, :].transpose(0, 3, 2, 1)
                X_all[:, 64 * h:64 * h + 64, slots, :] = sub

    xpad = np.zeros((B, 128, 130, 2, 130), dtype=NPF8)
    xp = np.pad(x, ((0, 0), (0, 0), (1, 1), (1, 1))).astype(NPF8)
    for e in range(2):
        xpad[:, :, :, e, :] = xp[:, e * 128:(e + 1) * 128]

    wc = w_comp[:, :, 0, 0].astype(np.float64)
    W2 = np.einsum('tkij,kc->tijc', w_enc.astype(np.float64), wc)
    bias2 = b_enc.astype(np.float64) + \
        w_enc.astype(np.float64).sum(axis=(2, 3)) @ b_comp.astype(np.float64)
    amax = max(np.abs(W2).max(), 1e-30)
    SCALE = 2.0 ** np.floor(np.log2(192.0 / amax))
    w2s = np.zeros((128, 18, 25), dtype=NPF8)
    for di in range(3):
        for dj in range(3):
            for e in range(2):
                w2s[:, (di * 3 + dj) * 2 + e, :] = \
                    (W2[:, di, dj, e * 128:(e + 1) * 128].T * SCALE).astype(NPF8)
    b2 = np.zeros((128, 1), dtype=np.float32)
    sc = np.zeros((128, 1), dtype=np.float32)
    idn = np.zeros((128, 25), dtype=NPBF)
    for jo in range(4):
        b2[32 * jo:32 * jo + 25, 0] = (pe * bias2).astype(np.float32)
        sc[32 * jo:32 * jo + 25, 0] = pe / SCALE
        idn[32 * jo + np.arange(25), np.arange(25)] = 1

    shm = np.zeros((128, 3, 128), dtype=NPBF)
    shm[np.arange(128), 0, np.arange(128)] = 1
    shm[np.arange(127), 1, np.arange(127) + 1] = 1
    shm[63, 1, 64] = 0
    shm[np.arange(1, 128), 2, np.arange(1, 128) - 1] = 1
    shm[64, 2, 63] = 0

    in_maps = []
    for b in range(B):
        in_maps.append({
            "xall": np.ascontiguousarray(X_all[b]),
            "xcp": np.ascontiguousarray(xpad[b]),
            "w2": w2s, "b2": b2, "sc": sc, "idn": idn, "shm": shm,
        })
    return in_maps


def kernel(x, w_comp, b_comp, w_enc, b_enc, power_p):
    x = np.asarray(x, dtype=np.float32)
    in_maps = _host_prep(x, np.asarray(w_comp), np.asarray(b_comp),
                         np.asarray(w_enc), np.asarray(b_enc),
                         np.asarray(power_p))
    nc = _get_nc()
    res = run_bass_kernel_spmd(nc, in_maps, list(range(NCORES)))
    global LAST_RESULTS
    LAST_RESULTS = res
    outs = np.stack([np.asarray(res.results[i]["out"]) for i in range(NCORES)])
    o = outs.reshape(B, 2, 64, NB, C).astype(np.float32)
    out = np.zeros((B, C, HP, WP), dtype=np.float32)
    for h in range(2):
        out[:, :, 32 * h:32 * h + 32, :] = o[:, h].transpose(0, 3, 2, 1)
    return np.ascontiguousarray(out)
